# revision 1
# baseline (speedup 1.0000x reference)
"""BertSelfAttention TRN2 Bass kernel (8-core data-parallel over batch).

Per core (one batch element):
  hidden [T, H] -> hT via PE transposes -> fused QKV projection split by
  orientation (qT/kT feature-major, v token-major), then per-head attention
  entirely in k-on-partitions layout:
    pass 1: raw scores (mask folded in via an aux contraction row) -> exp ->
            column sums S1 via ones-matmul -> c = -ln(S1)  (a valid per-query
            softmax stabilizer: max <= ln S1 <= max + ln(T))
    pass 2: scores recomputed with c injected via a second aux contraction
            row -> exp(8*(s+c)) which is the softmax numerator up to a
            per-query factor -> context matmul with [v | 1] stationary gives
            unnormalized context and the normalizer Z in one accumulation ->
            PE transpose back to [q, d] -> multiply by 1/Z.
All matmuls run in float32r (fast PE mode, fp32 PSUM accumulation).
"""
import sys

sys.path.insert(0, "/opt/trn_rl_repo")

import contextlib

import numpy as np
import concourse.bacc as bacc
import concourse.mybir as mybir
import concourse.tile as tile
from concourse.bass_utils import run_bass_kernel_spmd

F32 = mybir.dt.float32
F32R = mybir.dt.float32r
EXP = mybir.ActivationFunctionType.Exp
LN = mybir.ActivationFunctionType.Ln

HD = 64  # head dim (fixed)


@contextlib.contextmanager
def _single_act_table():
    """During compile, resolve Exp and Ln only from the one table set that
    holds both, so the kernel loads activation tables once instead of
    thrashing between exp_and_others and natural_log_exp_and_others."""
    orig = bacc.get_activation_tables

    def patched(arch):
        tables = orig(arch)
        if "natural_log_exp_and_others" in tables:
            for name, fns in tables.items():
                if name != "natural_log_exp_and_others":
                    fns.discard(mybir.ActivationFunctionType.Exp)
                    fns.discard(mybir.ActivationFunctionType.Ln)
        return tables

    bacc.get_activation_tables = patched
    try:
        yield
    finally:
        bacc.get_activation_tables = orig


def build_module(T, H, NH):
    """One-core program; run SPMD on 8 cores with per-core batch slices."""
    NT = T // 128      # token tiles
    NHT = H // 128     # hidden-dim tiles
    QC = min(512, T)   # query chunk (moving-dim >= 256 keeps f32r at full rate)
    NQC = T // QC

    nc = bacc.Bacc("TRN2", target_bir_lowering=False, debug=False, num_devices=8)

    hidden = nc.dram_tensor("hidden", [T, H], F32R, kind="ExternalInput").ap()
    w = nc.dram_tensor("w", [H, 3 * H], F32R, kind="ExternalInput").ap()
    mask_row = nc.dram_tensor("mask_row", [1, T], F32R, kind="ExternalInput").ap()
    ones_row = nc.dram_tensor("ones_row", [1, T], F32R, kind="ExternalInput").ap()
    neg_row = nc.dram_tensor("neg_row", [1, T], F32R, kind="ExternalInput").ap()
    ones_col = nc.dram_tensor("ones_col", [128, 1], F32R, kind="ExternalInput").ap()
    ones_blk = nc.dram_tensor("ones_blk", [128, NT * NH], F32R, kind="ExternalInput").ap()
    qk_bias = nc.dram_tensor("qk_bias", [128, NH], F32, kind="ExternalInput").ap()
    v_bias = nc.dram_tensor("v_bias", [128, H], F32, kind="ExternalInput").ap()
    ident_r = nc.dram_tensor("ident_r", [128, 128], F32R, kind="ExternalInput").ap()
    ident_f = nc.dram_tensor("ident_f", [128, 128], F32, kind="ExternalInput").ap()
    out = nc.dram_tensor("out", [T, H], F32, kind="ExternalOutput").ap()

    with tile.TileContext(nc) as tc:
        with tc.tile_pool(name="persist", bufs=1) as persist, tc.tile_pool(
            name="work", bufs=2
        ) as work, tc.tile_pool(name="outp", bufs=4) as outp, tc.tile_pool(
            name="psb", bufs=1, space="PSUM"
        ) as psb, tc.tile_pool(name="psc", bufs=4, space="PSUM") as psc, tc.tile_pool(
            name="pss", bufs=2, space="PSUM"
        ) as pss:
            # ---- constants ----
            idr = persist.tile([128, 128], F32R, tag="idr")
            idf = persist.tile([128, 128], F32, tag="idf")
            nc.sync.dma_start(out=idr, in_=ident_r)
            nc.sync.dma_start(out=idf, in_=ident_f)
            onec = persist.tile([128, 1], F32R, tag="onec")
            nc.sync.dma_start(out=onec, in_=ones_col)
            qkb = persist.tile([128, NH], F32, tag="qkb")
            nc.sync.dma_start(out=qkb, in_=qk_bias)
            vb = persist.tile([128, H], F32, tag="vb")
            nc.sync.dma_start(out=vb, in_=v_bias)

            # ---- phase 0: hT[p, ht, t] = hidden[t, ht*128+p] ----
            hT = persist.tile([128, NHT, T], F32R, tag="hT")
            for t in range(NT):
                hid = work.tile([128, H], F32R, tag="hid")
                nc.sync.dma_start(out=hid, in_=hidden[t * 128 : (t + 1) * 128, :])
                for hb in range(NHT):
                    tp = pss.tile([128, 128], F32R, tag="small")
                    nc.tensor.transpose(tp[:], hid[:, hb * 128 : (hb + 1) * 128], idr[:])
                    nc.vector.tensor_copy(hT[:, hb, t * 128 : (t + 1) * 128], tp[:])

            # ---- phase 1: v_aug[p, kt, h, 0:64] = v proj + bias; [.., 64] = 1 ----
            wv = persist.tile([128, NHT, H], F32R, tag="wv")
            for ht in range(NHT):
                wsl = w[ht * 128 : (ht + 1) * 128, :].rearrange(
                    "p (h three d) -> p h three d", three=3, d=HD
                )
                nc.sync.dma_start(
                    out=wv[:, ht, :].rearrange("p (h d) -> p h d", d=HD),
                    in_=wsl[:, :, 2, :],
                )
            v_aug = persist.tile([128, NT, NH, HD + 1], F32R, tag="v_aug")
            nc.sync.dma_start(
                out=v_aug[:, :, :, HD : HD + 1],
                in_=ones_blk.rearrange("p (a b one) -> p a b one", b=NH, one=1),
            )
            VW = min(512, H)
            NVH = VW // HD
            for t in range(NT):
                for half in range(H // VW):
                    vp = psc.tile([128, VW], F32, tag="sc")
                    for ht in range(NHT):
                        nc.tensor.matmul(
                            vp[:],
                            hT[:, ht, t * 128 : (t + 1) * 128],
                            wv[:, ht, half * VW : (half + 1) * VW],
                            start=(ht == 0),
                            stop=(ht == NHT - 1),
                        )
                    nc.vector.tensor_add(
                        v_aug[:, t, half * NVH : (half + 1) * NVH, 0:HD],
                        vp[:].rearrange("p (h d) -> p h d", d=HD),
                        vb[:, half * VW : (half + 1) * VW].rearrange(
                            "p (h d) -> p h d", d=HD
                        ),
                    )

            # ---- per-head attention ----
            for h in range(NH):
                # fused q|k projection for this head: psum [128f(q0:64,k64:128), T]
                wqk = work.tile([128, NHT, 128], F32R, tag="wqk")
                nc.sync.dma_start(
                    out=wqk,
                    in_=w[:, h * 3 * HD : h * 3 * HD + 128].rearrange(
                        "(ht p) f -> p ht f", p=128
                    ),
                )
                qkp = psb.tile([128, T], F32, tag="pj")
                for ht in range(NHT):
                    for half in range(NQC):
                        nc.tensor.matmul(
                            qkp[:, half * QC : (half + 1) * QC],
                            wqk[:, ht, :],
                            hT[:, ht, half * QC : (half + 1) * QC],
                            start=(ht == 0),
                            stop=(ht == NHT - 1),
                        )
                qaux = work.tile([66, T], F32R, tag="qaux")
                nc.vector.tensor_scalar_add(qaux[0:64, :], qkp[0:64, :], qkb[0:64, h : h + 1])
                ktmp = work.tile([128, T], F32R, tag="ktmp")
                nc.vector.tensor_scalar_add(
                    ktmp[64:128, :], qkp[64:128, :], qkb[64:128, h : h + 1]
                )
                kaux = work.tile([66, T], F32R, tag="kaux")
                nc.sync.dma_start(out=kaux[0:64, :], in_=ktmp[64:128, :])
                nc.sync.dma_start(out=kaux[64:65, :], in_=mask_row)
                # row 65 pairs with qaux's c row (= +ln S1_half); -2 injects
                # -2*ln(S1_half), a valid stabilizer in [max, max+2*ln T]
                nc.sync.dma_start(out=kaux[65:66, :], in_=neg_row)
                nc.sync.dma_start(out=qaux[64:65, :], in_=ones_row)

                # pass 1: S1 sums -> c = -ln(S1) per query
                csb = work.tile([1, T], F32R, tag="csb")
                for qc in range(NQC):
                    qs = qaux[0:65, qc * QC : (qc + 1) * QC]
                    s1p = pss.tile([1, QC], F32, tag="small")
                    for kt in range(NT):
                        sp = psc.tile([128, QC], F32, tag="sc")
                        e1 = work.tile([128, QC], F32R, tag="e1", bufs=3)
                        nc.tensor.matmul(
                            sp[:],
                            kaux[0:65, kt * 128 : (kt + 1) * 128],
                            qs,
                            start=True,
                            stop=True,
                        )
                        # half-scale keeps S1 <= e^29.5, inside ACT Ln's
                        # valid input range (Ln breaks above ~2^64)
                        nc.scalar.activation(out=e1[:], in_=sp[:], func=EXP, scale=0.5)
                        nc.tensor.matmul(
                            s1p[:],
                            onec[:],
                            e1[:],
                            start=(kt == 0),
                            stop=(kt == NT - 1),
                        )
                    nc.scalar.activation(
                        out=csb[:, qc * QC : (qc + 1) * QC],
                        in_=s1p[:],
                        func=LN,
                        scale=1.0,
                    )
                nc.sync.dma_start(out=qaux[65:66, :], in_=csb[:])

                # pass 2: e8 = exp(8*(raw - 1e4*m - lnS1)) ; ctx/Z accumulate
                ctxT = work.tile([65, T], F32, tag="ctxT")
                for qc in range(NQC):
                    qs = qaux[0:66, qc * QC : (qc + 1) * QC]
                    cxp = pss.tile([65, QC], F32, tag="small")
                    for kt in range(NT):
                        sp = psc.tile([128, QC], F32, tag="sc")
                        e8 = work.tile([128, QC], F32R, tag="e8", bufs=3)
                        nc.tensor.matmul(
                            sp[:],
                            kaux[0:66, kt * 128 : (kt + 1) * 128],
                            qs,
                            start=True,
                            stop=True,
                        )
                        nc.scalar.activation(out=e8[:], in_=sp[:], func=EXP, scale=8.0)
                        nc.tensor.matmul(
                            cxp[:],
                            v_aug[:, kt, h, :],
                            e8[:],
                            start=(kt == 0),
                            stop=(kt == NT - 1),
                        )
                    nc.vector.tensor_copy(ctxT[:, qc * QC : (qc + 1) * QC], cxp[:])

                # transpose back per query tile, normalize by Z, write out
                for qt in range(NT):
                    trp = pss.tile([128, 65], F32, tag="small")
                    nc.tensor.transpose(
                        trp[:], ctxT[:, qt * 128 : (qt + 1) * 128], idf[0:65, 0:65]
                    )
                    rz = outp.tile([128, 1], F32, tag="rz")
                    nc.vector.reciprocal(rz[:], trp[:, 64:65])
                    ot = outp.tile([128, HD], F32, tag="ot")
                    nc.vector.tensor_scalar_mul(ot[:], trp[:, 0:64], rz[:])
                    nc.sync.dma_start(
                        out=out[qt * 128 : (qt + 1) * 128, h * HD : (h + 1) * HD],
                        in_=ot,
                    )

    with _single_act_table():
        nc.compile()
    return nc


_module_cache = {}


def _get_module(T, H, NH):
    key = (T, H, NH)
    if key not in _module_cache:
        _module_cache[key] = build_module(T, H, NH)
    return _module_cache[key]


def run_sharded(hidden_states, attention_mask, w_qkv, b_qkv, trace=False):
    B, T, H = hidden_states.shape
    NH = H // HD
    NT = T // 128
    nc = _get_module(T, H, NH)

    w_np = np.ascontiguousarray(w_qkv.astype(np.float32))
    b_np = np.asarray(b_qkv, dtype=np.float32)
    # qk_bias[p, h] = b[h*192 + p]  (q bias rows 0-63, k bias rows 64-127)
    qkb = np.empty((128, NH), np.float32)
    for h in range(NH):
        qkb[:, h] = b_np[h * 3 * HD : h * 3 * HD + 128]
    # v_bias broadcast [128, H]
    vb_row = np.empty((H,), np.float32)
    for h in range(NH):
        vb_row[h * HD : (h + 1) * HD] = b_np[h * 3 * HD + 2 * HD : h * 3 * HD + 3 * HD]
    vb = np.broadcast_to(vb_row, (128, H)).copy()
    ones_row = np.ones((1, T), np.float32)
    neg_row = np.full((1, T), -2.0, np.float32)
    ones_col = np.ones((128, 1), np.float32)
    ones_blk = np.ones((128, NT * NH), np.float32)
    ident = np.eye(128, dtype=np.float32)

    in_maps = []
    for b in range(B):
        m = np.asarray(attention_mask[b]).reshape(-1).astype(np.float32)
        in_maps.append(
            dict(
                hidden=np.ascontiguousarray(hidden_states[b].astype(np.float32)),
                w=w_np,
                mask_row=(m * np.float32(-10000.0)).reshape(1, T),
                ones_row=ones_row,
                neg_row=neg_row,
                ones_col=ones_col,
                ones_blk=ones_blk,
                qk_bias=qkb,
                v_bias=vb,
                ident_r=ident,
                ident_f=ident,
            )
        )
    res = run_bass_kernel_spmd(nc, in_maps, core_ids=list(range(B)), trace=trace)
    return np.stack([res.results[b]["out"] for b in range(B)]), res


def kernel(hidden_states, attention_mask, w_qkv, b_qkv):
    out, _ = run_sharded(
        np.asarray(hidden_states),
        np.asarray(attention_mask),
        np.asarray(w_qkv),
        np.asarray(b_qkv),
    )
    return out.astype(np.float32)

